# revision 1
# baseline (speedup 1.0000x reference)
"""Trainium2 Bass kernel for nn_BasicDNC (4-layer transformer + external
memory read + tied LM head), SPMD over 8 NeuronCores.

Sharding:
  - tokens (B*T = 4096) split 512/core; cores 0-3 own batch 0, 4-7 batch 1
  - attention K/V allgathered within each 4-core batch group
  - memory bank + lm head token-sharded (full mem_K^T / tok_embed^T per core)

Layouts: activations kept transposed ([d, tok]) so every matmul contracts
over the partition dim; scores / logits come out in [tok, free] layout.
"""
import sys

sys.path.insert(0, "/opt/trn_rl_repo")

import numpy as np
import ml_dtypes

import concourse.bass as bass
import concourse.bacc as bacc
import concourse.mybir as mybir
import concourse.tile as tile
from concourse.bass_utils import run_bass_kernel_spmd
from concourse.masks import make_identity

F32 = mybir.dt.float32
BF16 = mybir.dt.bfloat16
FP16 = mybir.dt.float16
U8 = mybir.dt.uint8
U16 = mybir.dt.uint16
U32 = mybir.dt.uint32
I32 = mybir.dt.int32
AF = mybir.ActivationFunctionType
ALU = mybir.AluOpType
AX = mybir.AxisListType

N_CORES = 8
P = 128
D = 512
H = 8
DH = 64
L = 4
FF = 2048
B = 2
T = 2048
TOK = 512          # tokens per core
NT = TOK // P      # token tiles per core (4)
ND = D // P        # d tiles (4)
NFF = FF // P      # ff tiles (16)
NK = T // P        # key tiles per batch group (16)
SLOTS = 32768
SC = SLOTS // 2    # max-op chunk (16384)
V_SIZE = 32000
TOPK = 8
EPS = 1e-8
VSW = 65           # per-head v-store width (64 v cols + 1 ones col)
SSH = SLOTS // N_CORES   # slots per core (4096)
VSH = V_SIZE // N_CORES  # vocab per core (4000)


def build_program(flags):
    nc = bacc.Bacc(None, num_devices=N_CORES)

    # ---------------- io ----------------
    x0T_d = nc.dram_tensor("x0T", [D, TOK], F32, kind="ExternalInput")
    posT_d = nc.dram_tensor("posT", [D, TOK], F32, kind="ExternalInput")
    wq_d = nc.dram_tensor("wq", [L, D, D], BF16, kind="ExternalInput")
    wk_d = nc.dram_tensor("wk", [L, D, D], BF16, kind="ExternalInput")
    wv_d = nc.dram_tensor("wv", [L, D, D], BF16, kind="ExternalInput")
    wo_d = nc.dram_tensor("wo", [L, D, D], BF16, kind="ExternalInput")
    w1_d = nc.dram_tensor("w1", [L, D, FF], BF16, kind="ExternalInput")
    w2_d = nc.dram_tensor("w2", [L, FF, D], BF16, kind="ExternalInput")
    wqm_d = nc.dram_tensor("wqm", [D, D], BF16, kind="ExternalInput")
    wr_d = nc.dram_tensor("wr", [D, D], BF16, kind="ExternalInput")
    mkT_d = nc.dram_tensor("mkT", [D, SSH], BF16, kind="ExternalInput")
    mv_d = nc.dram_tensor("mv", [SLOTS, D], BF16, kind="ExternalInput")
    eT_d = nc.dram_tensor("eT", [D, VSH], BF16, kind="ExternalInput")
    if flags["bias"]:
        bo_d = nc.dram_tensor("bo", [L, D], F32, kind="ExternalInput")
        b1_d = nc.dram_tensor("b1", [L, FF], F32, kind="ExternalInput")
        b2_d = nc.dram_tensor("b2", [L, D], F32, kind="ExternalInput")
        bqm_d = nc.dram_tensor("bqm", [D], F32, kind="ExternalInput")
        br_d = nc.dram_tensor("br", [D], F32, kind="ExternalInput")
    if flags["normw"]:
        n1_d = nc.dram_tensor("n1", [L, D], F32, kind="ExternalInput")
        n2_d = nc.dram_tensor("n2", [L, D], F32, kind="ExternalInput")
        no_d = nc.dram_tensor("no", [D], F32, kind="ExternalInput")
    if flags["salience"]:
        sal_d = nc.dram_tensor("sal", [1, SSH], F32, kind="ExternalInput")

    logits_d = nc.dram_tensor("logits", [B * T, VSH], FP16, kind="ExternalOutput")
    debug = flags.get("debug", False)
    if debug:
        dbg_emb = nc.dram_tensor("dbg_emb", [D, TOK], F32, kind="ExternalOutput")
        dbg_lyr = nc.dram_tensor("dbg_lyr", [L, D, TOK], F32, kind="ExternalOutput")
        dbg_mqT = nc.dram_tensor("dbg_mqT", [D, TOK], F32, kind="ExternalOutput")
        dbg_v8 = nc.dram_tensor("dbg_v8", [TOK, 8], F32, kind="ExternalOutput")
        dbg_idx = nc.dram_tensor("dbg_idx", [TOK, 8], U32, kind="ExternalOutput")
        dbg_read = nc.dram_tensor("dbg_read", [TOK, D], F32, kind="ExternalOutput")
        dbg_xout = nc.dram_tensor("dbg_xout", [D, TOK], F32, kind="ExternalOutput")
        dbg_kf = nc.dram_tensor("dbg_kf", [D, 4 * TOK], F32, kind="ExternalOutput")
        dbg_ao = nc.dram_tensor("dbg_ao", [H * DH, TOK], F32, kind="ExternalOutput")
        dbg_vf = nc.dram_tensor("dbg_vf", [4 * TOK, H * VSW], F32, kind="ExternalOutput")
        dbg_den = nc.dram_tensor("dbg_den", [H, TOK], F32, kind="ExternalOutput")
        dbg_pex = nc.dram_tensor("dbg_pex", [P, TOK], F32, kind="ExternalOutput")

    groups = [[0, 1, 2, 3], [4, 5, 6, 7]]
    ablate = flags.get("ablate", frozenset())

    import contextlib

    with tile.TileContext(nc) as tc, contextlib.ExitStack() as ctx:
        persist = ctx.enter_context(tc.tile_pool(name="persist", bufs=1))
        consts = ctx.enter_context(tc.tile_pool(name="consts", bufs=1))
        dram = ctx.enter_context(tc.tile_pool(name="dram", bufs=2, space="DRAM"))
        psum_s = ctx.enter_context(tc.tile_pool(name="psum_s", bufs=4, space="PSUM"))
        psum_a = ctx.enter_context(tc.tile_pool(name="psum_a", bufs=1, space="PSUM"))
        small = ctx.enter_context(tc.tile_pool(name="small", bufs=2))

        ones_bf = consts.tile([P, 1], BF16, tag="ones_bf", name="ones_bf")
        nc.vector.memset(ones_bf[:], 1.0)
        eps_t = consts.tile([1, 1], F32, tag="eps_t", name="eps_t")
        nc.vector.memset(eps_t[:], EPS)

        # persistent residual stream (transposed): xT fp32 + bf16 shadow
        xT = [persist.tile([P, TOK], F32, tag=f"xT{t}", name=f"xT{t}") for t in range(ND)]
        xq = [persist.tile([P, TOK], BF16, tag=f"xq{t}", name=f"xq{t}") for t in range(ND)]

        def rms_norm(tiles_f32, tiles_bf16, normw_ap):
            """In-place RMS norm over d (partition dim x ND tiles) of the
            [d, tok] stream; refresh bf16 shadow."""
            ssq = psum_a.tile([1, TOK], F32, tag="ssq", name="ssq")
            for t in range(ND):
                sq = small.tile([P, TOK], BF16, tag="sq", name="sq")
                nc.scalar.activation(sq[:], tiles_f32[t][:], AF.Square)
                nc.tensor.matmul(ssq[:], lhsT=ones_bf[:], rhs=sq[:],
                                 start=(t == 0), stop=(t == ND - 1))
            scale1 = small.tile([1, TOK], F32, tag="scale1", name="scale1")
            nc.scalar.activation(scale1[:], ssq[:], AF.Sqrt,
                                 bias=eps_t[:, 0:1], scale=1.0 / D)
            scaleb = small.tile([P, TOK], F32, tag="scaleb", name="scaleb")
            nc.gpsimd.partition_broadcast(scaleb[:], scale1[:])
            nc.vector.reciprocal(scaleb[:], scaleb[:])
            for t in range(ND):
                nc.vector.tensor_mul(tiles_f32[t][:], tiles_f32[t][:], scaleb[:])
                if normw_ap is not None:
                    nw = small.tile([P, 1], F32, tag="nw", name="nw")
                    nc.sync.dma_start(nw[:], normw_ap[t * P:(t + 1) * P, None])
                    nc.vector.tensor_scalar(tiles_f32[t][:], tiles_f32[t][:],
                                            nw[:, 0:1], scalar2=None, op0=ALU.mult)
                nc.scalar.activation(tiles_bf16[t][:], tiles_f32[t][:], AF.Copy)

        def load_bias_tile(pool, ap_1d, t, tag):
            bt = pool.tile([P, 1], F32, tag=tag)
            nc.sync.dma_start(bt[:], ap_1d[t * P:(t + 1) * P, None])
            return bt

        # ---------------- embedding ----------------
        for t in range(ND):
            p0 = small.tile([P, TOK], F32, tag="emb0", name="emb0")
            p1 = small.tile([P, TOK], F32, tag="emb1", name="emb1")
            nc.sync.dma_start(p0[:], x0T_d[t * P:(t + 1) * P, :])
            nc.sync.dma_start(p1[:], posT_d[t * P:(t + 1) * P, :])
            nc.vector.tensor_add(xT[t][:], p0[:], p1[:])
            nc.scalar.activation(xq[t][:], xT[t][:], AF.Copy)
            if debug:
                nc.sync.dma_start(dbg_emb[t * P:(t + 1) * P, :], xT[t][:])

        # ---------------- transformer layers ----------------
        n_layers = 0 if "layers" in ablate else L
        with (
            tc.tile_pool(name="wts", bufs=1) as wts,
            tc.tile_pool(name="attn", bufs=1) as attn,
            tc.tile_pool(name="psb", bufs=2) as psb,
            tc.tile_pool(name="ffh", bufs=1) as ffh,
        ):
            for l in range(n_layers):
                # --- QKV projections ---
                wq_sb = [wts.tile([P, D], BF16, tag=f"wq{c}", name=f"wq{c}") for c in range(ND)]
                wk_sb = [wts.tile([P, D], BF16, tag=f"wk{c}", name=f"wk{c}") for c in range(ND)]
                wv_sb = [wts.tile([P, D], BF16, tag=f"wv{c}", name=f"wv{c}") for c in range(ND)]
                wo_sb = [wts.tile([DH, D], BF16, tag=f"wo{c}", name=f"wo{c}") for c in range(H)]
                for c in range(ND):
                    nc.sync.dma_start(wq_sb[c][:], wq_d[l, c * P:(c + 1) * P, :])
                    nc.sync.dma_start(wk_sb[c][:], wk_d[l, c * P:(c + 1) * P, :])
                    nc.sync.dma_start(wv_sb[c][:], wv_d[l, c * P:(c + 1) * P, :])
                for c in range(H):
                    nc.sync.dma_start(wo_sb[c][:], wo_d[l, c * DH:(c + 1) * DH, :])

                qT = [attn.tile([P, TOK], BF16, tag=f"qT{m}", name=f"qT{m}") for m in range(ND)]
                kT = [attn.tile([P, TOK], BF16, tag=f"kT{m}", name=f"kT{m}") for m in range(ND)]
                vst = [attn.tile([P, H, VSW], BF16, tag=f"vst{m}", name=f"vst{m}") for m in range(NT)]

                ag_in = dram.tile([D, TOK + H * VSW], BF16, tag="ag_in", name="ag_in")
                ag_out = dram.tile([4 * D, TOK + H * VSW], BF16, tag="ag_out", name="ag_out")

                for m in range(ND):
                    ps = psum_s.tile([P, TOK], F32, tag="ps", name="ps")
                    for c in range(ND):
                        nc.tensor.matmul(ps[:], lhsT=wq_sb[c][:, m * P:(m + 1) * P],
                                         rhs=xq[c][:], start=(c == 0), stop=(c == ND - 1))
                    nc.scalar.activation(qT[m][:], ps[:], AF.Copy)
                    ps = psum_s.tile([P, TOK], F32, tag="ps", name="ps")
                    for c in range(ND):
                        nc.tensor.matmul(ps[:], lhsT=wk_sb[c][:, m * P:(m + 1) * P],
                                         rhs=xq[c][:], start=(c == 0), stop=(c == ND - 1))
                    nc.scalar.activation(kT[m][:], ps[:], AF.Copy)
                    nc.sync.dma_start(ag_in[m * P:(m + 1) * P, 0:TOK], kT[m][:])
                for m in range(NT):
                    ps = psum_s.tile([P, TOK], F32, tag="ps", name="ps")
                    for c in range(ND):
                        nc.tensor.matmul(ps[:], lhsT=xq[c][:, m * P:(m + 1) * P],
                                         rhs=wv_sb[c][:], start=(c == 0), stop=(c == ND - 1))
                    pv = ps[:].rearrange("p (h d) -> p h d", h=H)
                    nc.vector.tensor_copy(vst[m][:, :, 0:DH], pv[:])
                    nc.vector.memset(vst[m][:, :, DH:VSW], 1.0)
                    nc.sync.dma_start(
                        ag_in[m * P:(m + 1) * P, TOK:].rearrange(
                            "p (h w) -> p h w", h=H),
                        vst[m][:])

                nc.gpsimd.collective_compute(
                    "AllGather", ALU.bypass, replica_groups=groups,
                    ins=[ag_in.opt()], outs=[ag_out.opt()])

                ago = ag_out[:].rearrange("(r x) c -> r x c", r=4)
                kfull = [attn.tile([P, 4 * TOK], BF16, tag=f"kfull{t}", name=f"kfull{t}")
                         for t in range(ND)]
                for t in range(ND):
                    nc.sync.dma_start(
                        kfull[t][:].rearrange("p (r t) -> p r t", r=4),
                        ago[:, t * P:(t + 1) * P, 0:TOK].rearrange("r p t -> p r t"))
                vfull = [attn.tile([P, H, VSW], BF16, tag=f"vfull{t}", name=f"vfull{t}")
                         for t in range(NK)]
                for t in range(NK):
                    r, rr = t // NT, t % NT
                    nc.sync.dma_start(
                        vfull[t][:],
                        ago[r, rr * P:(rr + 1) * P, TOK:].rearrange(
                            "p (h w) -> p h w", h=H))

                if debug and l == 0:
                    for t in range(ND):
                        nc.gpsimd.dma_start(dbg_kf[t * P:(t + 1) * P, :], kfull[t][:])
                    for t in range(NK):
                        nc.gpsimd.dma_start(
                            dbg_vf[t * P:(t + 1) * P, :],
                            vfull[t][:].rearrange("p h w -> p (h w)"))

                # --- attention (scores kept transposed [k, q]) ---
                # per-head AO in [64, TOK] tiles at base partition 0; the
                # O-projection contracts per-head (C=64) so no restacking.
                ao_h = [attn.tile([DH, TOK], BF16, tag=f"ao_h{h}", name=f"ao_h{h}")
                        for h in range(H)]
                for h in range(H):
                    dt, lo = h // 2, (h % 2) * DH
                    pall = psb.tile([P, NK, TOK], BF16, tag="pexp", name="pexp")
                    for kt in range(NK):
                        ps = psum_s.tile([P, TOK], F32, tag="ps", name="ps")
                        nc.tensor.matmul(
                            ps[:],
                            lhsT=kfull[dt][lo:lo + DH, kt * P:(kt + 1) * P],
                            rhs=qT[dt][lo:lo + DH, :], start=True, stop=True)
                        nc.scalar.activation(pall[:, kt, :], ps[:], AF.Exp,
                                             scale=DH ** -0.5)
                        if debug and l == 0 and h == 0 and kt == 0:
                            nc.gpsimd.dma_start(dbg_pex[:], pall[:, 0, :])
                    aops = psum_a.tile([P, TOK], F32, tag=f"ao{h % 2}", name=f"ao{h % 2}")
                    for kt in range(NK):
                        nc.tensor.matmul(aops[0:VSW, :], lhsT=vfull[kt][:, h, :],
                                         rhs=pall[:, kt, :],
                                         start=(kt == 0), stop=(kt == NK - 1))
                    # per-head denominator: row DH -> bcast -> recip -> normalize
                    den1 = attn.tile([P, TOK], F32, tag="den1", name="den1")
                    nc.scalar.activation(den1[DH:DH + 1, :], aops[DH:DH + 1, :],
                                         AF.Copy)
                    den0 = attn.tile([1, TOK], F32, tag="den0", name="den0")
                    nc.sync.dma_start(den0[:], den1[DH:DH + 1, :])
                    denb = attn.tile([P, TOK], F32, tag="denb", name="denb")
                    nc.gpsimd.partition_broadcast(denb[:], den0[:])
                    if debug and l == 0:
                        nc.gpsimd.dma_start(dbg_den[h:h + 1, :], den1[DH:DH + 1, :])
                    nc.vector.reciprocal(denb[0:DH, :], denb[0:DH, :])
                    nc.vector.tensor_mul(ao_h[h][:], aops[0:DH, :], denb[0:DH, :])
                    if debug and l == 0:
                        nc.gpsimd.dma_start(dbg_ao[h * DH:(h + 1) * DH, :], ao_h[h][:])

                # --- output projection + residual + norm1 ---
                for m in range(ND):
                    ps = psum_s.tile([P, TOK], F32, tag="ps", name="ps")
                    for h in range(H):
                        nc.tensor.matmul(
                            ps[:], lhsT=wo_sb[h][:, m * P:(m + 1) * P],
                            rhs=ao_h[h][:], start=(h == 0), stop=(h == H - 1))
                    nc.vector.tensor_add(xT[m][:], xT[m][:], ps[:])
                    if flags["bias"]:
                        bt = load_bias_tile(small, bo_d[l], m, "bo")
                        nc.vector.tensor_scalar(xT[m][:], xT[m][:], bt[:, 0:1],
                                                scalar2=None, op0=ALU.add)
                rms_norm(xT, xq, n1_d[l] if flags["normw"] else None)

                # --- FFN + residual + norm2 ---
                w1_sb = [wts.tile([P, FF], BF16, tag=f"w1_{c}", name=f"w1_{c}") for c in range(ND)]
                w2_sb = [wts.tile([P, D], BF16, tag=f"w2_{c}", name=f"w2_{c}") for c in range(NFF)]
                for c in range(ND):
                    nc.sync.dma_start(w1_sb[c][:], w1_d[l, c * P:(c + 1) * P, :])
                for c in range(NFF):
                    nc.sync.dma_start(w2_sb[c][:], w2_d[l, c * P:(c + 1) * P, :])
                hT = [ffh.tile([P, TOK], BF16, tag=f"hT{f}", name=f"hT{f}") for f in range(NFF)]
                for f in range(NFF):
                    ps = psum_s.tile([P, TOK], F32, tag="ps", name="ps")
                    for c in range(ND):
                        nc.tensor.matmul(ps[:], lhsT=w1_sb[c][:, f * P:(f + 1) * P],
                                         rhs=xq[c][:], start=(c == 0), stop=(c == ND - 1))
                    if flags["bias"]:
                        bt = load_bias_tile(small, b1_d[l], f, "b1")
                        nc.scalar.activation(hT[f][:], ps[:], AF.Gelu_apprx_tanh,
                                             bias=bt[:, 0:1])
                    else:
                        nc.scalar.activation(hT[f][:], ps[:], AF.Gelu_apprx_tanh)
                for m in range(ND):
                    ps = psum_s.tile([P, TOK], F32, tag="ps", name="ps")
                    for c in range(NFF):
                        nc.tensor.matmul(ps[:], lhsT=w2_sb[c][:, m * P:(m + 1) * P],
                                         rhs=hT[c][:], start=(c == 0), stop=(c == NFF - 1))
                    nc.vector.tensor_add(xT[m][:], xT[m][:], ps[:])
                    if flags["bias"]:
                        bt = load_bias_tile(small, b2_d[l], m, "b2")
                        nc.vector.tensor_scalar(xT[m][:], xT[m][:], bt[:, 0:1],
                                                scalar2=None, op0=ALU.add)
                rms_norm(xT, xq, n2_d[l] if flags["normw"] else None)
                if debug:
                    for t in range(ND):
                        nc.sync.dma_start(dbg_lyr[l, t * P:(t + 1) * P, :], xT[t][:])

        # ---------------- external memory read ----------------
        xoutq = [persist.tile([P, TOK], BF16, tag=f"xoq{t}", name=f"xoq{t}") for t in range(ND)]
        if "mem" in ablate:
            for t in range(ND):
                nc.vector.tensor_copy(xoutq[t][:], xq[t][:])
        if "mem" not in ablate:
         with (
            tc.tile_pool(name="mem", bufs=1) as mem,
            tc.tile_pool(name="scpool", bufs=1) as scpool,
            tc.tile_pool(name="kch", bufs=2) as kch,
            tc.tile_pool(name="msmall", bufs=2) as msmall,
            tc.tile_pool(name="gat", bufs=1) as gat,
        ):
            # mq^T = wqm^T x  (scaled by D^-0.5 on copy)
            wqm_sb = [mem.tile([P, D], BF16, tag=f"wqm{c}", name=f"wqm{c}") for c in range(ND)]
            for c in range(ND):
                nc.sync.dma_start(wqm_sb[c][:], wqm_d[c * P:(c + 1) * P, :])
            mqT = [mem.tile([P, TOK], BF16, tag=f"mqT{m}", name=f"mqT{m}") for m in range(ND)]
            for m in range(ND):
                ps = psum_s.tile([P, TOK], F32, tag="ps", name="ps")
                for c in range(ND):
                    nc.tensor.matmul(ps[:], lhsT=wqm_sb[c][:, m * P:(m + 1) * P],
                                     rhs=xq[c][:], start=(c == 0), stop=(c == ND - 1))
                if flags["bias"]:
                    bt = load_bias_tile(msmall, bqm_d, m, "bqm")
                    sc1 = msmall.tile([P, 1], F32, tag="bqms", name="bqms")
                    nc.vector.tensor_scalar(sc1[:], bt[:], float(D ** -0.5),
                                            scalar2=None, op0=ALU.mult)
                    nc.scalar.activation(mqT[m][:], ps[:], AF.Identity,
                                         bias=sc1[:, 0:1], scale=float(D ** -0.5))
                else:
                    nc.scalar.activation(mqT[m][:], ps[:], AF.Copy,
                                         scale=float(D ** -0.5))
                if debug:
                    nc.gpsimd.dma_start(dbg_mqT[m * P:(m + 1) * P, :], mqT[m][:])

            # --- allgather mq over all 8 cores (slot-sharded scoring) ---
            NTT = (B * T) // P          # 32 token tiles over all tokens
            NSC = SSH // TOK            # 8 slot chunks of 512
            mq_in = dram.tile([D, TOK], BF16, tag="mq_in", name="mq_in")
            mq_out = dram.tile([N_CORES * D, TOK], BF16, tag="mq_out", name="mq_out")
            for m in range(ND):
                nc.sync.dma_start(mq_in[m * P:(m + 1) * P, :], mqT[m][:])
            nc.gpsimd.collective_compute(
                "AllGather", ALU.bypass, replica_groups=[list(range(N_CORES))],
                ins=[mq_in.opt()], outs=[mq_out.opt()])
            mqo = mq_out[:].rearrange("(r x) c -> r x c", r=N_CORES)
            mqall = [mem.tile([P, B * T], BF16, tag=f"mqa{c}", name=f"mqa{c}")
                     for c in range(ND)]
            for c in range(ND):
                nc.sync.dma_start(
                    mqall[c][:].rearrange("p (r t) -> p r t", r=N_CORES),
                    mqo[:, c * P:(c + 1) * P, :].rearrange("r p t -> p r t"))

            # resident mem_K^T shard [D, SSH]
            mk_sb = [mem.tile([P, SSH], BF16, tag=f"mk{c}", name=f"mk{c}")
                     for c in range(ND)]
            for c in range(ND):
                nc.sync.dma_start(mk_sb[c][:], mkT_d[c * P:(c + 1) * P, :])

            if flags["salience"]:
                salb = mem.tile([P, SSH], BF16, tag="salb", name="salb")
                sal_sb = mem.tile([1, SSH], F32, tag="sal1", name="sal1")
                nc.sync.dma_start(sal_sb[:], sal_d[:])
                nc.gpsimd.partition_broadcast(salb[:], sal_sb[:])

            iota64i = consts.tile([P, 64], I32, tag="iota_i", name="iota_i")
            nc.gpsimd.iota(iota64i[:], pattern=[[1, 64]], base=0,
                           channel_multiplier=0)
            iota64f = consts.tile([P, 64], F32, tag="iota_f", name="iota_f")
            nc.vector.tensor_copy(iota64f[:], iota64i[:])

            read_bf = [mem.tile([P, D], BF16, tag=f"read{t}", name=f"read{t}") for t in range(NT)]

            # --- score all tokens against my slot shard; local top-8 ---
            cval = mem.tile([P, NTT, 8], FP16, tag="cval", name="cval")
            cidx = mem.tile([P, NTT, 8], U16, tag="cidx", name="cidx")
            if "scores" in ablate:
                for t in range(NT):
                    nc.vector.memset(read_bf[t][:], 0.0)
            for tt in range(0 if "scores" in ablate else NTT):
                sct = scpool.tile([P, SSH], FP16, tag="sct", name="sct")
                for s in range(NSC):
                    ps = psum_s.tile([P, TOK], F32, tag="ps", name="ps")
                    for c in range(ND):
                        nc.tensor.matmul(ps[:],
                                         lhsT=mqall[c][:, tt * P:(tt + 1) * P],
                                         rhs=mk_sb[c][:, s * TOK:(s + 1) * TOK],
                                         start=(c == 0), stop=(c == ND - 1))
                    if flags["salience"]:
                        nc.vector.tensor_add(
                            sct[:, s * TOK:(s + 1) * TOK], ps[:],
                            salb[:, s * TOK:(s + 1) * TOK])
                    else:
                        nc.scalar.activation(
                            sct[:, s * TOK:(s + 1) * TOK], ps[:], AF.Copy)
                nc.vector.max(out=cval[:, tt, :], in_=sct[:])
                nc.vector.max_index(out=cidx[:, tt, :], in_max=cval[:, tt, :],
                                    in_values=sct[:])

            if "scores" not in ablate and "topk" not in ablate:
                # --- all-to-all: each core gets its own tokens' 64 candidates ---
                cv_in = dram.tile([B * T, 8], FP16, tag="cv_in", name="cv_in")
                ci_in = dram.tile([B * T, 8], U16, tag="ci_in", name="ci_in")
                cv_out = dram.tile([B * T, 8], FP16, tag="cv_out", name="cv_out")
                ci_out = dram.tile([B * T, 8], U16, tag="ci_out", name="ci_out")
                nc.sync.dma_start(
                    cv_in[:].rearrange("(tt p) k -> p tt k", p=P), cval[:])
                nc.sync.dma_start(
                    ci_in[:].rearrange("(tt p) k -> p tt k", p=P), cidx[:])
                nc.gpsimd.collective_compute(
                    "AllToAll", ALU.bypass, replica_groups=[list(range(N_CORES))],
                    ins=[cv_in.opt()], outs=[cv_out.opt()])
                nc.gpsimd.collective_compute(
                    "AllToAll", ALU.bypass, replica_groups=[list(range(N_CORES))],
                    ins=[ci_in.opt()], outs=[ci_out.opt()])
                cvo = cv_out[:].rearrange("(r lp) k -> r lp k", r=N_CORES)
                cio = ci_out[:].rearrange("(r lp) k -> r lp k", r=N_CORES)

                for t in range(NT):
                    v64 = msmall.tile([P, 64], FP16, tag="v64", name="v64")
                    i64 = msmall.tile([P, 64], U16, tag="i64", name="i64")
                    nc.sync.dma_start(
                        v64[:].rearrange("p (r k) -> p r k", r=N_CORES),
                        cvo[:, t * P:(t + 1) * P, :].rearrange("r p k -> p r k"))
                    nc.sync.dma_start(
                        i64[:].rearrange("p (r k) -> p r k", r=N_CORES),
                        cio[:, t * P:(t + 1) * P, :].rearrange("r p k -> p r k"))
                    # global candidate indices (+ r*SSH per rank block), f32
                    cidxf = msmall.tile([P, 64], F32, tag="cidxf", name="cidxf")
                    nc.vector.tensor_copy(cidxf[:], i64[:])
                    for r in range(1, N_CORES):
                        nc.vector.tensor_scalar(
                            cidxf[:, r * 8:(r + 1) * 8], cidxf[:, r * 8:(r + 1) * 8],
                            float(r * SSH), scalar2=None, op0=ALU.add)
                    v8 = msmall.tile([P, 8], FP16, tag="v8", name="v8")
                    nc.vector.max(out=v8[:], in_=v64[:])
                    pos = msmall.tile([P, 8], U16, tag="pos", name="pos")
                    nc.vector.max_index(out=pos[:], in_max=v8[:], in_values=v64[:])
                    posf = msmall.tile([P, 8], F32, tag="posf", name="posf")
                    nc.vector.tensor_copy(posf[:], pos[:])
                    eq = msmall.tile([P, 8, 64], F32, tag="eq", name="eq")
                    nc.vector.tensor_tensor(
                        out=eq[:], in0=posf[:, :, None].to_broadcast([P, 8, 64]),
                        in1=iota64f[:, None, :].to_broadcast([P, 8, 64]),
                        op=ALU.is_equal)
                    nc.vector.tensor_tensor(
                        out=eq[:], in0=eq[:],
                        in1=cidxf[:, None, :].to_broadcast([P, 8, 64]),
                        op=ALU.mult)
                    idxf = msmall.tile([P, 8], F32, tag="idxf", name="idxf")
                    nc.vector.reduce_sum(idxf[:], eq[:], axis=AX.X)
                    idxu = msmall.tile([P, 8], U32, tag="idxu", name="idxu")
                    nc.vector.tensor_copy(idxu[:], idxf[:])
                    # softmax over the 8 scores
                    vf = msmall.tile([P, 8], F32, tag="vf", name="vf")
                    nc.vector.tensor_copy(vf[:], v8[:])
                    ew = msmall.tile([P, 8], F32, tag="ew", name="ew")
                    nc.scalar.activation(ew[:], vf[:], AF.Exp)
                    ssum = msmall.tile([P, 1], F32, tag="ssum", name="ssum")
                    nc.vector.reduce_sum(ssum[:], ew[:], axis=AX.X)
                    rs = msmall.tile([P, 1], F32, tag="rs", name="rs")
                    nc.vector.reciprocal(rs[:], ssum[:])
                    w8 = msmall.tile([P, 8], F32, tag="w8", name="w8")
                    nc.vector.tensor_scalar(w8[:], ew[:], rs[:, 0:1],
                                            scalar2=None, op0=ALU.mult)
                    # gather mem_V rows and weighted-sum them
                    vsel = gat.tile([P, TOPK, D], BF16, tag="vsel", name="vsel")
                    for k in range(TOPK):
                        nc.gpsimd.indirect_dma_start(
                            out=vsel[:, k, :], out_offset=None, in_=mv_d[:],
                            in_offset=bass.IndirectOffsetOnAxis(
                                ap=idxu[:, k:k + 1], axis=0),
                            bounds_check=SLOTS - 1, oob_is_err=False)
                    racc = msmall.tile([P, D], F32, tag="racc", name="racc")
                    rtmp = msmall.tile([P, D], F32, tag="rtmp", name="rtmp")
                    nc.vector.tensor_scalar(racc[:], vsel[:, 0, :], w8[:, 0:1],
                                            scalar2=None, op0=ALU.mult)
                    for k in range(1, TOPK):
                        nc.vector.tensor_scalar(rtmp[:], vsel[:, k, :], w8[:, k:k + 1],
                                                scalar2=None, op0=ALU.mult)
                        nc.vector.tensor_add(racc[:], racc[:], rtmp[:])
                    nc.vector.tensor_copy(read_bf[t][:], racc[:])
                    if debug:
                        nc.sync.dma_start(dbg_read[t * P:(t + 1) * P, :], racc[:])
                        nc.sync.dma_start(dbg_idx[t * P:(t + 1) * P, :], idxu[:])
                        nc.gpsimd.dma_start(dbg_v8[t * P:(t + 1) * P, :], v8[:])
            elif "topk" in ablate:
                for t in range(NT):
                    nc.vector.memset(read_bf[t][:], 0.0)
                    nc.vector.tensor_scalar(
                        read_bf[t][:, 0:1], cval[:, 0, 0:1], 0.0,
                        scalar2=None, op0=ALU.mult)

            # transpose read -> readT
            ident = consts.tile([P, P], BF16, tag="ident", name="ident")
            make_identity(nc, ident[:])
            readT = [mem.tile([P, TOK], BF16, tag=f"mqT{c}", name=f"readT{c}") for c in range(ND)]
            for t in range(NT):
                for c in range(ND):
                    pt = psum_s.tile([P, P], BF16, tag="ps", name="ps")
                    nc.tensor.transpose(pt[:], read_bf[t][:, c * P:(c + 1) * P],
                                        ident[:])
                    nc.scalar.activation(readT[c][:, t * P:(t + 1) * P], pt[:],
                                         AF.Copy)

            # x += read @ w_read ; final rms norm
            wr_sb = [mem.tile([P, D], BF16, tag=f"wqm{c}", name=f"wr{c}") for c in range(ND)]
            for c in range(ND):
                nc.sync.dma_start(wr_sb[c][:], wr_d[c * P:(c + 1) * P, :])
            for m in range(ND):
                ps = psum_s.tile([P, TOK], F32, tag="ps", name="ps")
                for c in range(ND):
                    nc.tensor.matmul(ps[:], lhsT=wr_sb[c][:, m * P:(m + 1) * P],
                                     rhs=readT[c][:], start=(c == 0), stop=(c == ND - 1))
                nc.vector.tensor_add(xT[m][:], xT[m][:], ps[:])
                if flags["bias"]:
                    bt = load_bias_tile(msmall, br_d, m, "br")
                    nc.vector.tensor_scalar(xT[m][:], xT[m][:], bt[:, 0:1],
                                            scalar2=None, op0=ALU.add)
            rms_norm(xT, xoutq, no_d if flags["normw"] else None)
            if debug:
                for t in range(ND):
                    nc.sync.dma_start(dbg_xout[t * P:(t + 1) * P, :], xT[t][:])

        # ---------------- lm head (tied embeddings, vocab-sharded) ----------------
        if "lm" not in ablate:
         with (
            tc.tile_pool(name="et", bufs=3) as etp,
            tc.tile_pool(name="lmp", bufs=1) as lmp,
            tc.tile_pool(name="lout", bufs=4) as lout,
        ):
            NTT = (B * T) // P
            xo_in = dram.tile([D, TOK], BF16, tag="xo_in", name="xo_in")
            xo_out = dram.tile([N_CORES * D, TOK], BF16, tag="xo_out", name="xo_out")
            for m in range(ND):
                nc.sync.dma_start(xo_in[m * P:(m + 1) * P, :], xoutq[m][:])
            nc.gpsimd.collective_compute(
                "AllGather", ALU.bypass, replica_groups=[list(range(N_CORES))],
                ins=[xo_in.opt()], outs=[xo_out.opt()])
            xoo = xo_out[:].rearrange("(r x) c -> r x c", r=N_CORES)
            xall = [lmp.tile([P, B * T], BF16, tag=f"xall{c}", name=f"xall{c}")
                    for c in range(ND)]
            for c in range(ND):
                nc.sync.dma_start(
                    xall[c][:].rearrange("p (r t) -> p r t", r=N_CORES),
                    xoo[:, c * P:(c + 1) * P, :].rearrange("r p t -> p r t"))

            VC = 512
            nvc = (VSH + VC - 1) // VC
            for v in range(nvc):
                vn = min(VC, VSH - v * VC)
                et_sb = etp.tile([P, ND, VC], BF16, tag="et", name="et")
                nc.sync.dma_start(
                    et_sb[:, :, :vn],
                    eT_d[:, v * VC:v * VC + vn].rearrange("(c p) t -> p c t", p=P))
                for t in range(NTT):
                    ps = psum_s.tile([P, VC], F32, tag="ps", name="ps")
                    for c in range(ND):
                        nc.tensor.matmul(ps[:, :vn],
                                         lhsT=xall[c][:, t * P:(t + 1) * P],
                                         rhs=et_sb[:, c, :vn],
                                         start=(c == 0), stop=(c == ND - 1))
                    lo = lout.tile([P, VC], FP16, tag="lo", name="lo")
                    nc.scalar.activation(lo[:, :vn], ps[:, :vn], AF.Copy)
                    nc.sync.dma_start(
                        logits_d[t * P:(t + 1) * P, v * VC:v * VC + vn],
                        lo[:, :vn])

    nc.compile()
    return nc


# ---------------------------------------------------------------------------
# host-side sharding / assembly
# ---------------------------------------------------------------------------

def prep_inputs(inputs):
    bf = ml_dtypes.bfloat16
    ids = np.asarray(inputs["input_ids"])
    tok = np.asarray(inputs["tok_embed"], np.float32)
    pos = np.asarray(inputs["pos_embed"], np.float32)

    flags = {
        "bias": not (
            np.all(np.asarray(inputs["blk_bo"]) == 0)
            and np.all(np.asarray(inputs["blk_ffb1"]) == 0)
            and np.all(np.asarray(inputs["blk_ffb2"]) == 0)
            and np.all(np.asarray(inputs["bq_mem"]) == 0)
            and np.all(np.asarray(inputs["b_read"]) == 0)
        ),
        "normw": not (
            np.all(np.asarray(inputs["blk_norm1"]) == 1)
            and np.all(np.asarray(inputs["blk_norm2"]) == 1)
            and np.all(np.asarray(inputs["norm_out_w"]) == 1)
        ),
        "salience": not np.all(np.asarray(inputs["salience"]) == 0),
    }

    shared = {
        "wq": np.ascontiguousarray(np.asarray(inputs["blk_wq"]).astype(bf)),
        "wk": np.ascontiguousarray(np.asarray(inputs["blk_wk"]).astype(bf)),
        "wv": np.ascontiguousarray(np.asarray(inputs["blk_wv"]).astype(bf)),
        "wo": np.ascontiguousarray(np.asarray(inputs["blk_wo"]).astype(bf)),
        "w1": np.ascontiguousarray(np.asarray(inputs["blk_ffw1"]).astype(bf)),
        "w2": np.ascontiguousarray(np.asarray(inputs["blk_ffw2"]).astype(bf)),
        "wqm": np.ascontiguousarray(np.asarray(inputs["wq_mem"]).astype(bf)),
        "wr": np.ascontiguousarray(np.asarray(inputs["w_read"]).astype(bf)),
        "mv": np.ascontiguousarray(np.asarray(inputs["mem_V"]).astype(bf)),
    }
    mkT_full = np.asarray(inputs["mem_K"], np.float32).T.astype(bf)
    eT_full = tok.T.astype(bf)
    if flags["bias"]:
        shared.update(
            bo=np.asarray(inputs["blk_bo"], np.float32),
            b1=np.asarray(inputs["blk_ffb1"], np.float32),
            b2=np.asarray(inputs["blk_ffb2"], np.float32),
            bqm=np.asarray(inputs["bq_mem"], np.float32),
            br=np.asarray(inputs["b_read"], np.float32),
        )
    if flags["normw"]:
        shared.update(
            n1=np.asarray(inputs["blk_norm1"], np.float32),
            n2=np.asarray(inputs["blk_norm2"], np.float32),
            no=np.asarray(inputs["norm_out_w"], np.float32),
        )
    sal_full = np.asarray(inputs["salience"], np.float32)

    in_maps = []
    for c in range(N_CORES):
        b, p0 = c // 4, (c % 4) * TOK
        ids_c = ids[b, p0:p0 + TOK].astype(np.int64)
        m = dict(shared)
        m["x0T"] = np.ascontiguousarray(tok[ids_c].T)
        m["posT"] = np.ascontiguousarray(pos[p0:p0 + TOK].T)
        m["mkT"] = np.ascontiguousarray(mkT_full[:, c * SSH:(c + 1) * SSH])
        m["eT"] = np.ascontiguousarray(eT_full[:, c * VSH:(c + 1) * VSH])
        if flags["salience"]:
            m["sal"] = np.ascontiguousarray(sal_full[None, c * SSH:(c + 1) * SSH])
        in_maps.append(m)
    return in_maps, flags


def assemble(results):
    parts = [np.asarray(results[c]["logits"], np.float32) for c in range(N_CORES)]
    full = np.concatenate(parts, axis=1)        # [4096, 32000]
    return full.reshape(B, T, V_SIZE)


_PROGRAM_CACHE = {}


def get_program(flags):
    key = tuple(sorted(flags.items()))
    if key not in _PROGRAM_CACHE:
        _PROGRAM_CACHE[key] = build_program(flags)
    return _PROGRAM_CACHE[key]


def kernel(**inputs):
    in_maps, flags = prep_inputs(inputs)
    nc = get_program(flags)
    res = run_bass_kernel_spmd(nc, in_maps, list(range(N_CORES)))
    return assemble(res.results)



# revision 13
# speedup vs baseline: 1.7852x; 1.7852x over previous
"""Trainium2 Bass kernel for nn_BasicDNC (4-layer transformer + external
memory read + tied LM head), SPMD over 8 NeuronCores.

Sharding (v2, collective-light):
  - tokens (B*T = 4096) split 512/core; cores 0-3 own batch 0, 4-7 batch 1
  - attention K/V allgathered within each 4-core batch group (only
    remaining collective; outputs in Shared address space)
  - memory scoring is token-local: every core streams the full mem_K^T
    from DRAM and scores its own 512 tokens against all 32768 slots
    (2 passes of 2 token tiles to shorten the merge tail)
  - top-8 via group-max hierarchy: 16-slot group maxima (DVE reduce),
    top-8 groups per token, re-gather those groups' raw scores from a
    DRAM spill, final top-8 + index recovery with f32 tie-breaking
  - lm head token-local: stream the full tied-embedding table, each core
    writes logits[512 tokens, 32000] fp16

Layouts: activations kept transposed ([d, tok]) so matmuls contract over
the partition dim; attention scores row-tiled in head pairs (C=64).
"""
import sys

sys.path.insert(0, "/opt/trn_rl_repo")

import numpy as np
import ml_dtypes

import concourse.bass as bass
import concourse.bacc as bacc
import concourse.mybir as mybir
import concourse.tile as tile
from concourse.bass_utils import run_bass_kernel_spmd
from concourse.masks import make_identity

F32 = mybir.dt.float32
BF16 = mybir.dt.bfloat16
FP16 = mybir.dt.float16
U16 = mybir.dt.uint16
U32 = mybir.dt.uint32
I32 = mybir.dt.int32
AF = mybir.ActivationFunctionType
ALU = mybir.AluOpType
AX = mybir.AxisListType

N_CORES = 8
P = 128
D = 512
H = 8
DH = 64
L = 4
FF = 2048
B = 2
T = 2048
TOK = 512          # tokens per core
NT = TOK // P      # token tiles per core (4)
ND = D // P        # d tiles (4)
NFF = FF // P      # ff tiles (16)
NK = T // P        # key tiles per batch group (16)
SLOTS = 32768
V_SIZE = 32000
TOPK = 8
EPS = 1e-8
VSW = 65           # per-head v-store width (64 v cols + 1 ones col)

GW = 16            # top-k group width (slots per group)
SC = 1024          # scoring slot chunk
NSC = SLOTS // SC  # 32 chunks
NG = SLOTS // GW   # 2048 groups per token
GPC = SC // GW     # 64 groups per chunk
VC = 512           # lm-head vocab chunk
NVC = (V_SIZE + VC - 1) // VC   # 63 (last chunk 256)


def build_program(flags):
    nc = bacc.Bacc(None, num_devices=N_CORES)

    # ---------------- io ----------------
    x0T_d = nc.dram_tensor("x0T", [D, TOK], F32, kind="ExternalInput")
    wq_d = nc.dram_tensor("wq", [L, D, D], BF16, kind="ExternalInput")
    wk_d = nc.dram_tensor("wk", [L, D, D], BF16, kind="ExternalInput")
    wv_d = nc.dram_tensor("wv", [L, D, D], BF16, kind="ExternalInput")
    wo_d = nc.dram_tensor("wo", [L, D, D], BF16, kind="ExternalInput")
    w1_d = nc.dram_tensor("w1", [L, D, FF], BF16, kind="ExternalInput")
    w2_d = nc.dram_tensor("w2", [L, FF, D], BF16, kind="ExternalInput")
    wqm_d = nc.dram_tensor("wqm", [D, D], BF16, kind="ExternalInput")
    wr_d = nc.dram_tensor("wr", [D, D], BF16, kind="ExternalInput")
    mkT_d = nc.dram_tensor("mkT", [D, SLOTS], BF16, kind="ExternalInput")
    mv_d = nc.dram_tensor("mv", [SLOTS, D], BF16, kind="ExternalInput")
    eT_d = nc.dram_tensor("eT", [D, V_SIZE], BF16, kind="ExternalInput")
    if flags["bias"]:
        bo_d = nc.dram_tensor("bo", [L, D], F32, kind="ExternalInput")
        b1_d = nc.dram_tensor("b1", [L, FF], F32, kind="ExternalInput")
        b2_d = nc.dram_tensor("b2", [L, D], F32, kind="ExternalInput")
        bqm_d = nc.dram_tensor("bqm", [D], F32, kind="ExternalInput")
        br_d = nc.dram_tensor("br", [D], F32, kind="ExternalInput")
    if flags["normw"]:
        n1_d = nc.dram_tensor("n1", [L, D], F32, kind="ExternalInput")
        n2_d = nc.dram_tensor("n2", [L, D], F32, kind="ExternalInput")
        no_d = nc.dram_tensor("no", [D], F32, kind="ExternalInput")
    if flags["salience"]:
        sal_d = nc.dram_tensor("sal", [1, SLOTS], F32, kind="ExternalInput")

    logits_d = nc.dram_tensor("logits", [TOK, V_SIZE], FP16, kind="ExternalOutput")
    debug = flags.get("debug", False)
    if debug:
        dbg_xout = nc.dram_tensor("dbg_xout", [D, TOK], F32, kind="ExternalOutput")
        dbg_idx = nc.dram_tensor("dbg_idx", [TOK, 8], U32, kind="ExternalOutput")
        dbg_v8 = nc.dram_tensor("dbg_v8", [TOK, 8], F32, kind="ExternalOutput")

    groups = [[0, 1, 2, 3], [4, 5, 6, 7]]

    import contextlib

    with tile.TileContext(nc) as tc, contextlib.ExitStack() as ctx:
        persist = ctx.enter_context(tc.tile_pool(name="persist", bufs=1))
        consts = ctx.enter_context(tc.tile_pool(name="consts", bufs=1))
        psq = ctx.enter_context(tc.tile_pool(name="psq", bufs=2, space="PSUM"))
        small = ctx.enter_context(tc.tile_pool(name="small", bufs=2))

        ones_bf = consts.tile([P, 1], BF16, tag="ones_bf", name="ones_bf")
        nc.vector.memset(ones_bf[:], 1.0)
        eps_t = consts.tile([1, 1], F32, tag="eps_t", name="eps_t")
        nc.vector.memset(eps_t[:], EPS)

        # persistent residual stream (transposed): xT fp32 + bf16 shadow
        xT = [persist.tile([P, TOK], F32, tag=f"xT{t}", name=f"xT{t}") for t in range(ND)]
        xq = [persist.tile([P, TOK], BF16, tag=f"xq{t}", name=f"xq{t}") for t in range(ND)]

        def rms_norm(tiles_f32, tiles_bf16, normw_ap):
            """In-place RMS norm over d (partition dim x ND tiles) of the
            [d, tok] stream; refresh bf16 shadow."""
            ssq = psq.tile([P, TOK], F32, tag="ps", name="ssq")
            for t in range(ND):
                sq = small.tile([P, TOK], BF16, tag="sq", name="sq")
                nc.vector.tensor_mul(sq[:], tiles_f32[t][:], tiles_f32[t][:])
                nc.tensor.matmul(ssq[0:1, :], lhsT=ones_bf[:], rhs=sq[:],
                                 start=(t == 0), stop=(t == ND - 1))
            scale1 = small.tile([1, TOK], F32, tag="scale1", name="scale1")
            nc.scalar.activation(scale1[:], ssq[0:1, :], AF.Sqrt,
                                 bias=eps_t[:, 0:1], scale=1.0 / D)
            scaleb = small.tile([P, TOK], F32, tag="scaleb", name="scaleb")
            nc.gpsimd.partition_broadcast(scaleb[:], scale1[:])
            nc.vector.reciprocal(scaleb[:], scaleb[:])
            for t in range(ND):
                nc.vector.tensor_mul(tiles_f32[t][:], tiles_f32[t][:], scaleb[:])
                if normw_ap is not None:
                    nw = small.tile([P, 1], F32, tag="nw", name="nw")
                    nc.sync.dma_start(nw[:], normw_ap[t * P:(t + 1) * P, None])
                    nc.vector.tensor_scalar(tiles_f32[t][:], tiles_f32[t][:],
                                            nw[:, 0:1], scalar2=None, op0=ALU.mult)
                nc.scalar.activation(tiles_bf16[t][:], tiles_f32[t][:], AF.Copy)

        def load_bias_tile(pool, ap_1d, t, tag):
            bt = pool.tile([P, 1], F32, tag=tag)
            nc.sync.dma_start(bt[:], ap_1d[t * P:(t + 1) * P, None])
            return bt

        # ---------------- embedding (pos folded in on host) ----------------
        for t in range(ND):
            nc.sync.dma_start(xT[t][:], x0T_d[t * P:(t + 1) * P, :])
            nc.scalar.activation(xq[t][:], xT[t][:], AF.Copy)

        # ---------------- transformer layers ----------------
        with (
            tc.tile_pool(name="wqkv", bufs=2) as wqkv,
            tc.tile_pool(name="wffp", bufs=1) as wffp,
            tc.tile_pool(name="attn", bufs=1) as attn,
            tc.tile_pool(name="pexp", bufs=3) as pexp,
            tc.tile_pool(name="ffh", bufs=1) as ffh,
            tc.tile_pool(name="ldram", bufs=2, space="DRAM") as ldram,
            tc.tile_pool(name="pss", bufs=2, space="PSUM") as pss,
            tc.tile_pool(name="pav", bufs=1, space="PSUM") as pav,
        ):
            for l in range(L):
                # --- QKV weights (double-buffered across layers) ---
                wq_sb = [wqkv.tile([P, D], BF16, tag=f"wq{c}", name=f"wq{c}") for c in range(ND)]
                wk_sb = [wqkv.tile([P, D], BF16, tag=f"wk{c}", name=f"wk{c}") for c in range(ND)]
                wv_sb = [wqkv.tile([P, D], BF16, tag=f"wv{c}", name=f"wv{c}") for c in range(ND)]
                wo_sb = [wqkv.tile([DH, D], BF16, tag=f"wo{h}", name=f"wo{h}") for h in range(H)]
                for c in range(ND):
                    nc.sync.dma_start(wq_sb[c][:], wq_d[l, c * P:(c + 1) * P, :])
                    nc.sync.dma_start(wk_sb[c][:], wk_d[l, c * P:(c + 1) * P, :])
                    nc.sync.dma_start(wv_sb[c][:], wv_d[l, c * P:(c + 1) * P, :])
                for h in range(H):
                    nc.sync.dma_start(wo_sb[h][:], wo_d[l, h * DH:(h + 1) * DH, :])

                qT = [attn.tile([P, TOK], BF16, tag=f"qT{m}", name=f"qT{m}") for m in range(ND)]
                kT = [attn.tile([P, TOK], BF16, tag=f"kT{m}", name=f"kT{m}") for m in range(ND)]
                vst = [attn.tile([P, H, VSW], BF16, tag=f"vst{m}", name=f"vst{m}") for m in range(NT)]

                ag_in = ldram.tile([D, TOK + H * VSW], BF16, tag="ag_in", name="ag_in")
                ag_out = ldram.tile([4 * D, TOK + H * VSW], BF16, tag="ag_out",
                                    name="ag_out")

                for m in range(ND):
                    ps = psq.tile([P, TOK], F32, tag="ps", name="ps")
                    for c in range(ND):
                        nc.tensor.matmul(ps[:], lhsT=wq_sb[c][:, m * P:(m + 1) * P],
                                         rhs=xq[c][:], start=(c == 0), stop=(c == ND - 1))
                    nc.scalar.activation(qT[m][:], ps[:], AF.Copy)
                    ps = psq.tile([P, TOK], F32, tag="ps", name="ps")
                    for c in range(ND):
                        nc.tensor.matmul(ps[:], lhsT=wk_sb[c][:, m * P:(m + 1) * P],
                                         rhs=xq[c][:], start=(c == 0), stop=(c == ND - 1))
                    nc.scalar.activation(kT[m][:], ps[:], AF.Copy)
                    nc.sync.dma_start(ag_in[m * P:(m + 1) * P, 0:TOK], kT[m][:])
                for m in range(NT):
                    ps = psq.tile([P, TOK], F32, tag="ps", name="ps")
                    for c in range(ND):
                        nc.tensor.matmul(ps[:], lhsT=xq[c][:, m * P:(m + 1) * P],
                                         rhs=wv_sb[c][:], start=(c == 0), stop=(c == ND - 1))
                    pv = ps[:].rearrange("p (h d) -> p h d", h=H)
                    nc.vector.tensor_copy(vst[m][:, :, 0:DH], pv[:])
                    nc.vector.memset(vst[m][:, :, DH:VSW], 1.0)
                    nc.sync.dma_start(
                        ag_in[m * P:(m + 1) * P, TOK:].rearrange(
                            "p (h w) -> p h w", h=H),
                        vst[m][:])

                nc.gpsimd.collective_compute(
                    "AllGather", ALU.bypass, replica_groups=groups,
                    ins=[ag_in.opt()], outs=[ag_out.opt()])

                ago = ag_out[:].rearrange("(r x) c -> r x c", r=4)
                kfull = [attn.tile([P, 4 * TOK], BF16, tag=f"kfull{t}", name=f"kfull{t}")
                         for t in range(ND)]
                for t in range(ND):
                    nc.sync.dma_start(
                        kfull[t][:].rearrange("p (r t) -> p r t", r=4),
                        ago[:, t * P:(t + 1) * P, 0:TOK].rearrange("r p t -> p r t"))
                vfull = [attn.tile([P, H, VSW], BF16, tag=f"vfull{t}", name=f"vfull{t}")
                         for t in range(NK)]
                for t in range(NK):
                    r, rr = t // NT, t % NT
                    nc.sync.dma_start(
                        vfull[t][:],
                        ago[r, rr * P:(rr + 1) * P, TOK:].rearrange(
                            "p (h w) -> p h w", h=H))

                # --- FFN weights (same-layer prefetch, single buffered) ---
                w1_sb = [wffp.tile([P, FF], BF16, tag=f"w1_{c}", name=f"w1_{c}") for c in range(ND)]
                w2_sb = [wffp.tile([P, D], BF16, tag=f"w2_{c}", name=f"w2_{c}") for c in range(NFF)]
                for c in range(ND):
                    nc.sync.dma_start(w1_sb[c][:], w1_d[l, c * P:(c + 1) * P, :])
                for c in range(NFF):
                    nc.sync.dma_start(w2_sb[c][:], w2_d[l, c * P:(c + 1) * P, :])

                # --- attention: head pairs, scores row-tiled (C=64), exp per
                # kt from a 2-bank psum span, av accumulated per kt ---
                ao_h = [attn.tile([DH, TOK], BF16, tag=f"ao_h{h}", name=f"ao_h{h}")
                        for h in range(H)]
                for i in range(H // 2):
                    # heads hA/hB occupy d-tile i, rows 0:64 / 64:128
                    hA, hB, dt = 2 * i, 2 * i + 1, i
                    avA = pav.tile([P, TOK], F32, tag="avA", name="avA")
                    avB = pav.tile([P, TOK], F32, tag="avB", name="avB")
                    for kt in range(NK):
                        pst = pss.tile([P, 2, TOK], F32, tag="pss", name="pst")
                        nc.tensor.matmul(
                            pst[:, 0, :],
                            lhsT=kfull[dt][0:DH, kt * P:(kt + 1) * P],
                            rhs=qT[dt][0:DH, :], start=True, stop=True,
                            tile_position=(0, 0))
                        nc.tensor.matmul(
                            pst[:, 1, :],
                            lhsT=kfull[dt][DH:P, kt * P:(kt + 1) * P],
                            rhs=qT[dt][DH:P, :], start=True, stop=True,
                            tile_position=(64, 0))
                        pexp_t = pexp.tile([P, 2, TOK], BF16, tag="pexp", name="pexp_t")
                        nc.scalar.activation(pexp_t[:], pst[:], AF.Exp,
                                             scale=DH ** -0.5)
                        nc.tensor.matmul(avA[0:VSW, :], lhsT=vfull[kt][:, hA, :],
                                         rhs=pexp_t[:, 0, :],
                                         start=(kt == 0), stop=(kt == NK - 1))
                        nc.tensor.matmul(avB[0:VSW, :], lhsT=vfull[kt][:, hB, :],
                                         rhs=pexp_t[:, 1, :],
                                         start=(kt == 0), stop=(kt == NK - 1))
                    # normalize by the ones-row denominator (row DH)
                    for h, av in ((hA, avA), (hB, avB)):
                        den1 = attn.tile([P, TOK], F32, tag="den1", name="den1")
                        nc.scalar.activation(den1[DH:DH + 1, :], av[DH:DH + 1, :],
                                             AF.Copy)
                        den0 = attn.tile([1, TOK], F32, tag="den0", name="den0")
                        nc.sync.dma_start(den0[:], den1[DH:DH + 1, :])
                        denb = attn.tile([DH, TOK], F32, tag="denb", name="denb")
                        nc.gpsimd.partition_broadcast(denb[:], den0[:])
                        nc.vector.reciprocal(denb[:], denb[:])
                        nc.vector.tensor_mul(ao_h[h][:], av[0:DH, :], denb[:])

                # --- output projection + residual + norm1 ---
                for m in range(ND):
                    ps = psq.tile([P, TOK], F32, tag="ps", name="ps")
                    for h in range(H):
                        nc.tensor.matmul(
                            ps[:], lhsT=wo_sb[h][:, m * P:(m + 1) * P],
                            rhs=ao_h[h][:],
                            start=(h == 0), stop=(h == H - 1))
                    nc.vector.tensor_add(xT[m][:], xT[m][:], ps[:])
                    if flags["bias"]:
                        bt = load_bias_tile(small, bo_d[l], m, "bo")
                        nc.vector.tensor_scalar(xT[m][:], xT[m][:], bt[:, 0:1],
                                                scalar2=None, op0=ALU.add)
                rms_norm(xT, xq, n1_d[l] if flags["normw"] else None)

                # --- FFN + residual + norm2 ---
                hT = [ffh.tile([P, TOK], BF16, tag=f"hT{f}", name=f"hT{f}") for f in range(NFF)]
                for f in range(NFF):
                    ps = psq.tile([P, TOK], F32, tag="ps", name="ps")
                    for c in range(ND):
                        nc.tensor.matmul(ps[:], lhsT=w1_sb[c][:, f * P:(f + 1) * P],
                                         rhs=xq[c][:], start=(c == 0), stop=(c == ND - 1))
                    if flags["bias"]:
                        bt = load_bias_tile(small, b1_d[l], f, "b1")
                        nc.scalar.activation(hT[f][:], ps[:], AF.Gelu_apprx_tanh,
                                             bias=bt[:, 0:1])
                    else:
                        nc.scalar.activation(hT[f][:], ps[:], AF.Gelu_apprx_tanh)
                for m in range(ND):
                    ps = psq.tile([P, TOK], F32, tag="ps", name="ps")
                    for c in range(NFF):
                        nc.tensor.matmul(ps[:], lhsT=w2_sb[c][:, m * P:(m + 1) * P],
                                         rhs=hT[c][:], start=(c == 0), stop=(c == NFF - 1))
                    nc.vector.tensor_add(xT[m][:], xT[m][:], ps[:])
                    if flags["bias"]:
                        bt = load_bias_tile(small, b2_d[l], m, "b2")
                        nc.vector.tensor_scalar(xT[m][:], xT[m][:], bt[:, 0:1],
                                                scalar2=None, op0=ALU.add)
                rms_norm(xT, xq, n2_d[l] if flags["normw"] else None)

        # ---------------- external memory read (token-local) ----------------
        xoutq = [persist.tile([P, TOK], BF16, tag=f"xoq{t}", name=f"xoq{t}") for t in range(ND)]
        with (
            tc.tile_pool(name="mem", bufs=1) as mem,
            tc.tile_pool(name="kch", bufs=3) as kch,
            tc.tile_pool(name="sctp", bufs=3) as sctp,
            tc.tile_pool(name="msmall", bufs=2) as msmall,
            tc.tile_pool(name="gat", bufs=2) as gat,
            tc.tile_pool(name="mdram", bufs=1, space="DRAM") as mdram,
            tc.tile_pool(name="psc", bufs=3, space="PSUM") as pscp,
        ):
            # mq^T = wqm^T x  (scaled by D^-0.5 on copy)
            wqm_sb = [mem.tile([P, D], BF16, tag=f"wqm{c}", name=f"wqm{c}") for c in range(ND)]
            for c in range(ND):
                nc.sync.dma_start(wqm_sb[c][:], wqm_d[c * P:(c + 1) * P, :])
            mqT = [mem.tile([P, TOK], BF16, tag=f"mqT{m}", name=f"mqT{m}") for m in range(ND)]
            for m in range(ND):
                ps = psq.tile([P, TOK], F32, tag="ps", name="ps")
                for c in range(ND):
                    nc.tensor.matmul(ps[:], lhsT=wqm_sb[c][:, m * P:(m + 1) * P],
                                     rhs=xq[c][:], start=(c == 0), stop=(c == ND - 1))
                if flags["bias"]:
                    bt = load_bias_tile(msmall, bqm_d, m, "bqm")
                    sc1 = msmall.tile([P, 1], F32, tag="bqms", name="bqms")
                    nc.vector.tensor_scalar(sc1[:], bt[:], float(D ** -0.5),
                                            scalar2=None, op0=ALU.mult)
                    nc.scalar.activation(mqT[m][:], ps[:], AF.Identity,
                                         bias=sc1[:, 0:1], scale=float(D ** -0.5))
                else:
                    nc.scalar.activation(mqT[m][:], ps[:], AF.Copy,
                                         scale=float(D ** -0.5))

            if flags["salience"]:
                salb = mem.tile([P, SLOTS], BF16, tag="salb", name="salb")
                sal_sb = mem.tile([1, SLOTS], F32, tag="sal1", name="sal1")
                nc.sync.dma_start(sal_sb[:], sal_d[:])
                nc.gpsimd.partition_broadcast(salb[:], sal_sb[:])

            # constants for the merge
            iota2k_i = msmall.tile([P, NG], I32, tag="iota2k_i", name="iota2k_i")
            nc.gpsimd.iota(iota2k_i[:], pattern=[[1, NG]], base=0,
                           channel_multiplier=0)
            bias2k = consts.tile([P, NG], F32, tag="bias2k", name="bias2k")
            nc.vector.tensor_copy(bias2k[:], iota2k_i[:])
            nc.vector.tensor_scalar(bias2k[:], bias2k[:], 1e-8,
                                    scalar2=None, op0=ALU.mult)
            iota16_i = consts.tile([P, GW], I32, tag="iota16_i", name="iota16_i")
            nc.gpsimd.iota(iota16_i[:], pattern=[[1, GW]], base=0,
                           channel_multiplier=0)
            iota16f = consts.tile([P, GW], F32, tag="iota16f", name="iota16f")
            nc.vector.tensor_copy(iota16f[:], iota16_i[:])

            sct_dram = mdram.tile([TOK, SLOTS], FP16, tag="sct_dram", name="sct_dram")
            gmax = [mem.tile([P, NG], F32, tag=f"gmax{t}", name=f"gmax{t}")
                    for t in range(NT)]
            read_bf = [mem.tile([P, D], BF16, tag=f"read{t}", name=f"read{t}")
                       for t in range(NT)]

            # row base offsets (token index * NG) per token tile, for the
            # score re-gather
            rowb = []
            for t in range(NT):
                rb = mem.tile([P, 1], I32, tag=f"rowb{t}", name=f"rowb{t}")
                nc.gpsimd.iota(rb[:], pattern=[[1, 1]], base=t * P * NG,
                               channel_multiplier=NG)
                rbf = mem.tile([P, 1], F32, tag=f"rowbf{t}", name=f"rowbf{t}")
                nc.vector.tensor_copy(rbf[:], rb[:])
                rowb.append(rbf)

            # --- scoring: 2 passes x 2 token tiles, stream mem_K chunks ---
            for pas in range(2):
                tts = [2 * pas, 2 * pas + 1]
                for s in range(NSC):
                    mk = kch.tile([P, ND, SC], BF16, tag="mk", name="mk")
                    nc.sync.dma_start(
                        mk[:], mkT_d[:, s * SC:(s + 1) * SC].rearrange(
                            "(c p) v -> p c v", p=P))
                    for tt in tts:
                        ps = pscp.tile([P, 2, TOK], F32, tag="psc", name="psc")
                        for half in range(2):
                            for c in range(ND):
                                nc.tensor.matmul(
                                    ps[:, half, :],
                                    lhsT=mqT[c][:, tt * P:(tt + 1) * P],
                                    rhs=mk[:, c, half * TOK:(half + 1) * TOK],
                                    start=(c == 0), stop=(c == ND - 1))
                        sct = sctp.tile([P, SC], FP16, tag="sct", name="sct")
                        if flags["salience"]:
                            nc.vector.tensor_add(
                                sct[:], ps[:].rearrange("p h t -> p (h t)"),
                                salb[:, s * SC:(s + 1) * SC])
                        else:
                            nc.scalar.activation(
                                sct[:], ps[:].rearrange("p h t -> p (h t)"),
                                AF.Copy)
                        nc.gpsimd.dma_start(
                            sct_dram[tt * P:(tt + 1) * P, s * SC:(s + 1) * SC],
                            sct[:])
                        nc.vector.reduce_max(
                            gmax[tt][:, s * GPC:(s + 1) * GPC],
                            sct[:].rearrange("p (g w) -> p g w", w=GW),
                            axis=AX.X)

                # --- merge per token tile of this pass ---
                for tt in tts:
                    nc.vector.tensor_add(gmax[tt][:], gmax[tt][:], bias2k[:])
                    g8 = msmall.tile([P, 8], F32, tag="g8", name="g8")
                    nc.vector.max(out=g8[:], in_=gmax[tt][:])
                    gpos = msmall.tile([P, 8], U16, tag="gpos", name="gpos")
                    nc.vector.max_index(out=gpos[:], in_max=g8[:],
                                        in_values=gmax[tt][:])
                    gposf = msmall.tile([P, 8], F32, tag="gposf", name="gposf")
                    nc.vector.tensor_copy(gposf[:], gpos[:])
                    # gather the 8 groups' raw scores from the spill
                    offf = msmall.tile([P, 8], F32, tag="offf", name="offf")
                    nc.vector.tensor_scalar(offf[:], gposf[:], rowb[tt][:, 0:1],
                                            scalar2=None, op0=ALU.add)
                    offu = msmall.tile([P, 8], U32, tag="offu", name="offu")
                    nc.vector.tensor_copy(offu[:], offf[:])
                    sg = gat.tile([P, 8, GW], FP16, tag="sg", name="sg")
                    sview = sct_dram[:].rearrange("t (g w) -> (t g) w", w=GW)
                    for k in range(TOPK):
                        nc.gpsimd.indirect_dma_start(
                            out=sg[:, k, :], out_offset=None, in_=sview,
                            in_offset=bass.IndirectOffsetOnAxis(
                                ap=offu[:, k:k + 1], axis=0),
                            bounds_check=TOK * NG - 1, oob_is_err=False)
                    sgf = msmall.tile([P, 8 * GW], F32, tag="sgf", name="sgf")
                    nc.vector.tensor_copy(sgf[:], sg[:].rearrange("p k w -> p (k w)"))
                    nc.vector.tensor_add(sgf[:], sgf[:], bias2k[:, 0:8 * GW])
                    v8 = msmall.tile([P, 8], F32, tag="v8", name="v8")
                    nc.vector.max(out=v8[:], in_=sgf[:])
                    # slot index = group_idx*16 + within-group position,
                    # recovered via equality match against biased scores
                    gelem = msmall.tile([P, 8, GW], F32, tag="gelem", name="gelem")
                    g16 = msmall.tile([P, 8], F32, tag="g16", name="g16")
                    nc.vector.tensor_scalar(g16[:], gposf[:], float(GW),
                                            scalar2=None, op0=ALU.mult)
                    nc.vector.tensor_tensor(
                        out=gelem[:], in0=g16[:, :, None].to_broadcast([P, 8, GW]),
                        in1=iota16f[:, None, :].to_broadcast([P, 8, GW]),
                        op=ALU.add)
                    eq = msmall.tile([P, 8, 8 * GW], F32, tag="eq", name="eq")
                    nc.vector.tensor_tensor(
                        out=eq[:], in0=v8[:, :, None].to_broadcast([P, 8, 8 * GW]),
                        in1=sgf[:, None, :].to_broadcast([P, 8, 8 * GW]),
                        op=ALU.is_equal)
                    nc.vector.tensor_tensor(
                        out=eq[:], in0=eq[:],
                        in1=gelem[:].rearrange("p k w -> p (k w)")[:, None, :]
                            .to_broadcast([P, 8, 8 * GW]),
                        op=ALU.mult)
                    idxf = msmall.tile([P, 8], F32, tag="idxf", name="idxf")
                    nc.vector.reduce_sum(idxf[:], eq[:], axis=AX.X)
                    idxu = msmall.tile([P, 8], U32, tag="idxu", name="idxu")
                    nc.vector.tensor_copy(idxu[:], idxf[:])
                    # softmax over the 8 scores
                    ew = msmall.tile([P, 8], F32, tag="ew", name="ew")
                    nc.scalar.activation(ew[:], v8[:], AF.Exp)
                    ssum = msmall.tile([P, 1], F32, tag="ssum", name="ssum")
                    nc.vector.reduce_sum(ssum[:], ew[:], axis=AX.X)
                    rs = msmall.tile([P, 1], F32, tag="rs", name="rs")
                    nc.vector.reciprocal(rs[:], ssum[:])
                    w8 = msmall.tile([P, 8], F32, tag="w8", name="w8")
                    nc.vector.tensor_scalar(w8[:], ew[:], rs[:, 0:1],
                                            scalar2=None, op0=ALU.mult)
                    # gather mem_V rows and weighted-sum them (tree reduce)
                    vsel = gat.tile([P, TOPK, D], BF16, tag="vsel", name="vsel")
                    for k in range(TOPK):
                        nc.gpsimd.indirect_dma_start(
                            out=vsel[:, k, :], out_offset=None, in_=mv_d[:],
                            in_offset=bass.IndirectOffsetOnAxis(
                                ap=idxu[:, k:k + 1], axis=0),
                            bounds_check=SLOTS - 1, oob_is_err=False)
                    racc = msmall.tile([P, D], F32, tag="racc", name="racc")
                    nc.vector.tensor_scalar(racc[:], vsel[:, 0, :], w8[:, 0:1],
                                            scalar2=None, op0=ALU.mult)
                    for k in range(1, TOPK):
                        nc.vector.scalar_tensor_tensor(
                            out=racc[:], in0=vsel[:, k, :], scalar=w8[:, k:k + 1],
                            in1=racc[:], op0=ALU.mult, op1=ALU.add)
                    nc.vector.tensor_copy(read_bf[tt][:], racc[:])
                    if debug:
                        nc.gpsimd.dma_start(dbg_idx[tt * P:(tt + 1) * P, :], idxu[:])
                        nc.gpsimd.dma_start(dbg_v8[tt * P:(tt + 1) * P, :], v8[:])

            # transpose read -> readT
            ident = consts.tile([P, P], BF16, tag="ident", name="ident")
            make_identity(nc, ident[:])
            readT = [mem.tile([P, TOK], BF16, tag=f"readT{c}", name=f"readT{c}")
                     for c in range(ND)]
            for t in range(NT):
                for c in range(ND):
                    pt = psq.tile([P, P], BF16, tag="ps", name="pt")
                    nc.tensor.transpose(pt[:], read_bf[t][:, c * P:(c + 1) * P],
                                        ident[:])
                    nc.scalar.activation(readT[c][:, t * P:(t + 1) * P], pt[:],
                                         AF.Copy)

            # x += read @ w_read ; final rms norm
            wr_sb = [mem.tile([P, D], BF16, tag=f"wr{c}", name=f"wr{c}") for c in range(ND)]
            for c in range(ND):
                nc.sync.dma_start(wr_sb[c][:], wr_d[c * P:(c + 1) * P, :])
            for m in range(ND):
                ps = psq.tile([P, TOK], F32, tag="ps", name="ps")
                for c in range(ND):
                    nc.tensor.matmul(ps[:], lhsT=wr_sb[c][:, m * P:(m + 1) * P],
                                     rhs=readT[c][:], start=(c == 0), stop=(c == ND - 1))
                nc.vector.tensor_add(xT[m][:], xT[m][:], ps[:])
                if flags["bias"]:
                    bt = load_bias_tile(msmall, br_d, m, "br")
                    nc.vector.tensor_scalar(xT[m][:], xT[m][:], bt[:, 0:1],
                                            scalar2=None, op0=ALU.add)
            rms_norm(xT, xoutq, no_d if flags["normw"] else None)
            if debug:
                for t in range(ND):
                    nc.sync.dma_start(dbg_xout[t * P:(t + 1) * P, :], xT[t][:])

        # ---------------- lm head (tied embeddings, token-local) ----------------
        with (
            tc.tile_pool(name="et", bufs=4) as etp,
            tc.tile_pool(name="lout", bufs=4) as lout,
            tc.tile_pool(name="psl", bufs=4, space="PSUM") as psl,
        ):
            for v in range(NVC):
                vn = min(VC, V_SIZE - v * VC)
                et_sb = etp.tile([P, ND, VC], BF16, tag="et", name="et")
                nc.sync.dma_start(
                    et_sb[:, :, :vn],
                    eT_d[:, v * VC:v * VC + vn].rearrange("(c p) t -> p c t", p=P))
                for t in range(NT):
                    ps = psl.tile([P, VC], F32, tag="lps", name="lps")
                    for c in range(ND):
                        nc.tensor.matmul(ps[:, :vn],
                                         lhsT=xoutq[c][:, t * P:(t + 1) * P],
                                         rhs=et_sb[:, c, :vn],
                                         start=(c == 0), stop=(c == ND - 1))
                    lo = lout.tile([P, VC], FP16, tag="lo", name="lo")
                    if t % 2 == 0:
                        nc.scalar.activation(lo[:, :vn], ps[:, :vn], AF.Copy)
                    else:
                        nc.vector.tensor_copy(lo[:, :vn], ps[:, :vn])
                    nc.sync.dma_start(
                        logits_d[t * P:(t + 1) * P, v * VC:v * VC + vn],
                        lo[:, :vn])

    nc.compile()
    return nc


# ---------------------------------------------------------------------------
# host-side sharding / assembly
# ---------------------------------------------------------------------------

def prep_inputs(inputs):
    bf = ml_dtypes.bfloat16
    ids = np.asarray(inputs["input_ids"])
    tok = np.asarray(inputs["tok_embed"], np.float32)
    pos = np.asarray(inputs["pos_embed"], np.float32)

    flags = {
        "bias": not (
            np.all(np.asarray(inputs["blk_bo"]) == 0)
            and np.all(np.asarray(inputs["blk_ffb1"]) == 0)
            and np.all(np.asarray(inputs["blk_ffb2"]) == 0)
            and np.all(np.asarray(inputs["bq_mem"]) == 0)
            and np.all(np.asarray(inputs["b_read"]) == 0)
        ),
        "normw": not (
            np.all(np.asarray(inputs["blk_norm1"]) == 1)
            and np.all(np.asarray(inputs["blk_norm2"]) == 1)
            and np.all(np.asarray(inputs["norm_out_w"]) == 1)
        ),
        "salience": not np.all(np.asarray(inputs["salience"]) == 0),
    }

    shared = {
        "wq": np.ascontiguousarray(np.asarray(inputs["blk_wq"]).astype(bf)),
        "wk": np.ascontiguousarray(np.asarray(inputs["blk_wk"]).astype(bf)),
        "wv": np.ascontiguousarray(np.asarray(inputs["blk_wv"]).astype(bf)),
        "wo": np.ascontiguousarray(np.asarray(inputs["blk_wo"]).astype(bf)),
        "w1": np.ascontiguousarray(np.asarray(inputs["blk_ffw1"]).astype(bf)),
        "w2": np.ascontiguousarray(np.asarray(inputs["blk_ffw2"]).astype(bf)),
        "wqm": np.ascontiguousarray(np.asarray(inputs["wq_mem"]).astype(bf)),
        "wr": np.ascontiguousarray(np.asarray(inputs["w_read"]).astype(bf)),
        "mv": np.ascontiguousarray(np.asarray(inputs["mem_V"]).astype(bf)),
        "mkT": np.ascontiguousarray(np.asarray(inputs["mem_K"], np.float32).T.astype(bf)),
        "eT": np.ascontiguousarray(tok.T.astype(bf)),
    }
    if flags["bias"]:
        shared.update(
            bo=np.asarray(inputs["blk_bo"], np.float32),
            b1=np.asarray(inputs["blk_ffb1"], np.float32),
            b2=np.asarray(inputs["blk_ffb2"], np.float32),
            bqm=np.asarray(inputs["bq_mem"], np.float32),
            br=np.asarray(inputs["b_read"], np.float32),
        )
    if flags["normw"]:
        shared.update(
            n1=np.asarray(inputs["blk_norm1"], np.float32),
            n2=np.asarray(inputs["blk_norm2"], np.float32),
            no=np.asarray(inputs["norm_out_w"], np.float32),
        )
    if flags["salience"]:
        shared["sal"] = np.ascontiguousarray(
            np.asarray(inputs["salience"], np.float32)[None, :])

    in_maps = []
    for c in range(N_CORES):
        b, p0 = c // 4, (c % 4) * TOK
        ids_c = ids[b, p0:p0 + TOK].astype(np.int64)
        m = dict(shared)
        m["x0T"] = np.ascontiguousarray((tok[ids_c] + pos[p0:p0 + TOK]).T)
        in_maps.append(m)
    return in_maps, flags


def assemble(results):
    parts = [np.asarray(results[c]["logits"], np.float32) for c in range(N_CORES)]
    full = np.concatenate(parts, axis=0)          # [4096, 32000]
    return full.reshape(B, T, V_SIZE)


_PROGRAM_CACHE = {}


def get_program(flags):
    key = tuple(sorted(flags.items()))
    if key not in _PROGRAM_CACHE:
        _PROGRAM_CACHE[key] = build_program(flags)
    return _PROGRAM_CACHE[key]


def kernel(**inputs):
    in_maps, flags = prep_inputs(inputs)
    nc = get_program(flags)
    res = run_bass_kernel_spmd(nc, in_maps, list(range(N_CORES)))
    return assemble(res.results)
